# revision 32
# baseline (speedup 1.0000x reference)
"""Trainium2 Bass kernel for hierarchical (word + sentence) additive attention
with ragged per-document flattening.

Data-parallel over the 32-document batch: 4 docs per NeuronCore x 8 cores.
All parameters are replicated (packed into one image -> one DMA). Inputs are
sharded on the host; encoder_feature is pre-transposed to [N, words] bf16 and
encoder_outputs cast to bf16. Each core computes its 4 docs' outputs
(c_t rows, per-sentence attention rows, sentence-attention rows); the host
concatenates and performs the ragged placement of attention values during
unsharding.
"""

import numpy as np
import ml_dtypes

import concourse.bass as bass
from concourse import mybir
from concourse.tile import TileContext
from concourse.bass_utils import run_bass_kernel_spmd

F32 = mybir.dt.float32
BF16 = mybir.dt.bfloat16
AF = mybir.ActivationFunctionType
ALU = mybir.AluOpType
AX = mybir.AxisListType

NCORES = 8
N = 512          # feature dim (2*hidden)
TK = 64          # max words per sentence
S = 40           # max sentences per doc
B_DOC = 32
NDL = B_DOC // NCORES      # docs per core = 4
NSL = NDL * S              # sentence slots per core = 160
WDOC = S * TK              # words per doc = 2560
NWL = NDL * WDOC           # words per core = 10240
NCH = N // 128             # 4 chunks of the feature dim
HALF_W = WDOC // 2         # 1280
SH = S // 2                # sentences per half doc = 20
GRP = 320                  # score-matmul word group (free dim)
NGRP = HALF_W // GRP       # 4 groups per half doc

# packed parameter images (f32), laid out per partition
# P1: word-level params (on the startup critical path)
_C_WD = 0
_C_ST = _C_WD + NCH * N           # 2048
_C_BD = _C_ST + NCH * NDL         # 2064
_C_V = _C_BD + NCH                # 2068
P1COLS = _C_V + NCH               # 2072
# P2: sentence-level params + identity
_C_WSD = 0
_C_SST = _C_WSD + NCH * N         # 2048
_C_BSD = _C_SST + NCH * NDL      # 2064
_C_SV = _C_BSD + NCH              # 2068
_C_SEF = _C_SV + NCH              # 2072
_C_SMASK = _C_SEF + NCH * NSL     # 2712
_C_IDENT = _C_SMASK + S           # 2752
P2COLS = _C_IDENT + 128           # 2880


def _patch_tile_drain():
    """This container's walrus build rejects >1 sync-wait on the Tile tail
    Drain (no-ctrl-struct instruction). Move the waits onto single-wait
    carrier NOPs on the sync engine."""
    import concourse.tile as tile_mod
    if getattr(tile_mod.TileContext, "_drain_patched", False):
        return
    from concourse.vector_clock import ScopedClock

    def _drain_and_barrier(self, tick_clock, wait_clock):
        nop = self.nc.sync.nop()
        wait_clock.add_sem_waits(nop.ins, ScopedClock({None: tick_clock.global_clock}))
        si = nop.ins.sync_info
        if si and si.on_wait and len(si.on_wait) > 1:
            extra = list(si.on_wait[1:])
            si.on_wait = si.on_wait[:1]
            for w in extra:
                n2 = self.nc.sync.nop()
                n2.ins.sync_info = mybir.SyncInfo(on_wait=[w], on_update=[])
        self.nc.sync.drain()
        self.nc.all_engine_barrier()
        popped = self.nc._tile_sem_poison_stack.pop()
        assert popped is self._sem_poison
        self.nc.clear_and_free_semaphores(list(self.sems.allocated().values()))
        self.nc.all_engine_barrier()

    tile_mod.TileContext._drain_and_barrier = _drain_and_barrier
    tile_mod.TileContext._drain_patched = True


def _split_sync_waits(nc, limit=1):
    """This container's walrus rejects instructions carrying more than `limit`
    sync waits. Hoist extra waits onto same-engine event-semaphore waits
    inserted just before the instruction (engine queues execute in order)."""
    for fn in nc.m.functions:
        for bb in fn.blocks:
            insts = bb.instructions
            i = 0
            while i < len(insts):
                inst = insts[i]
                si = inst.sync_info
                if si is not None and si.on_wait and len(si.on_wait) > limit:
                    waits = list(si.on_wait)
                    extra, keep = waits[:-limit], waits[-limit:]
                    for w in extra:
                        nop = mybir.InstEventSemaphore(
                            name=f"I-sw{nc.next_id()}",
                            engine=inst.engine,
                            sync_info=mybir.SyncInfo(on_wait=[w], on_update=[]),
                        )
                        insts.insert(i, nop)
                        i += 1
                    si.on_wait = keep
                i += 1


def build_nc():
    _patch_tile_drain()
    nc = bass.Bass(trn_type="TRN2")

    p1_d = nc.dram_tensor("p1", [128, P1COLS], F32, kind="ExternalInput")
    p2_d = nc.dram_tensor("p2", [128, P2COLS], F32, kind="ExternalInput")
    ef_T = nc.dram_tensor("ef_T", [N, NWL], BF16, kind="ExternalInput")
    eo = nc.dram_tensor("eo", [NWL, N], BF16, kind="ExternalInput")
    wmask_d = nc.dram_tensor("wmask", [SH, NDL * 2 * TK], F32, kind="ExternalInput")

    scr_s = nc.dram_tensor("scr_s", [1, NSL], F32, kind="Internal")
    scr_w = nc.dram_tensor("scr_w", [NDL * 2, HALF_W], F32, kind="Internal")

    out_ct = nc.dram_tensor("out_ct", [NDL, N], F32, kind="ExternalOutput")
    out_ma = nc.dram_tensor("out_ma", [NSL, TK], F32, kind="ExternalOutput")
    out_sattn = nc.dram_tensor("out_sattn", [NDL, S], F32, kind="ExternalOutput")

    with TileContext(nc) as tc:
        with (
            tc.tile_pool(name="singles", bufs=1) as SGL,
            tc.tile_pool(name="efp", bufs=4) as EFP,
            tc.tile_pool(name="thp", bufs=3) as THP,
            tc.tile_pool(name="eop", bufs=6) as EOP,
            tc.tile_pool(name="docp", bufs=3) as DOCP,
            tc.tile_pool(name="smalls", bufs=4) as SM,
            tc.tile_pool(name="srp", bufs=2) as SRP,
        ):
            # ---------- parameters: two packed images, two DMAs ----------
            P1 = SGL.tile([128, P1COLS], F32)
            nc.gpsimd.dma_start(out=P1, in_=p1_d[:, :])
            P2 = SGL.tile([128, P2COLS], F32)
            nc.gpsimd.dma_start(out=P2, in_=p2_d[:, :])
            M_img = SGL.tile([SH, NDL * 2 * TK], F32)
            nc.gpsimd.dma_start(out=M_img, in_=wmask_d[:, :])

            wd_sb = P1[:, _C_WD:_C_ST].rearrange("p (c n) -> p c n", n=N)
            sT_sb = P1[:, _C_ST:_C_BD].rearrange("p (c d) -> p c d", d=NDL)
            bd_sb = P1[:, _C_BD:_C_V]
            v_sb = P1[:, _C_V:P1COLS]
            wsd_sb = P2[:, _C_WSD:_C_SST].rearrange("p (c n) -> p c n", n=N)
            ssT_sb = P2[:, _C_SST:_C_BSD].rearrange("p (c d) -> p c d", d=NDL)
            bsd_sb = P2[:, _C_BSD:_C_SV]
            sv_sb = P2[:, _C_SV:_C_SEF]
            sefT_sb = P2[:, _C_SEF:_C_SMASK].rearrange("p (c s) -> p c s", s=NSL)
            smask_sb = P2[:4, _C_SMASK:_C_IDENT]
            ident = P2[:, _C_IDENT:P2COLS]

            v_bf = SGL.tile([128, NCH], BF16)
            nc.vector.tensor_copy(out=v_bf, in_=v_sb)

            # decoder features, transposed: decb[n_chunk][p, d] = (x @ W + b)[d, n]
            decb = SGL.tile([128, NCH, NDL], F32)
            sdecb = SGL.tile([128, NCH, NDL], F32)
            PP_cm = tc.tile_pool(name="pp1", bufs=1, space="PSUM")
            PP = PP_cm.__enter__()
            for w_sb, x_sb, b_sb, o_sb in (
                (wd_sb, sT_sb, bd_sb, decb),
                (wsd_sb, ssT_sb, bsd_sb, sdecb),
            ):
                for ci in range(NCH):
                    pdec = PP.tile([128, NDL], F32, tag="pdec")
                    for kc in range(NCH):
                        nc.tensor.matmul(
                            out=pdec[:, :],
                            lhsT=w_sb[:, kc, ci * 128:(ci + 1) * 128],
                            rhs=x_sb[:, kc, :],
                            start=(kc == 0),
                            stop=(kc == NCH - 1),
                        )
                    nc.vector.tensor_scalar_add(
                        out=o_sb[:, ci, :], in0=pdec[:, :], scalar1=b_sb[:, ci:ci + 1]
                    )

            # ---------- sentence-level attention ----------
            srow = SGL.tile([NDL, S], F32)
            stanh = SGL.tile([128, NCH, NSL], F32)
            for d in range(NDL):
                for ci in range(NCH):
                    nc.scalar.activation(
                        out=stanh[:, ci, d * S:(d + 1) * S],
                        in_=sefT_sb[:, ci, d * S:(d + 1) * S],
                        func=AF.Tanh,
                        bias=sdecb[:, ci, d:d + 1],
                    )
            pscs = PP.tile([1, NSL], F32, tag="pscs")
            for ci in range(NCH):
                nc.tensor.matmul(
                    out=pscs[:, :],
                    lhsT=sv_sb[:, ci:ci + 1],
                    rhs=stanh[:, ci, :],
                    start=(ci == 0),
                    stop=(ci == NCH - 1),
                )
            srow160 = SGL.tile([1, NSL], F32)
            nc.vector.tensor_copy(out=srow160[:, :], in_=pscs[0:1, :])
            nc.gpsimd.dma_start(out=scr_s[:, :], in_=srow160)
            nc.gpsimd.dma_start(
                out=srow[:, :], in_=scr_s.rearrange("o (d s) -> (o d) s", s=S))

            sneg = SM.tile([NDL, 1], F32, tag="sneg")
            nc.vector.tensor_reduce(out=sneg, in_=srow, axis=AX.X, op=ALU.max, negate=True)
            sexp = SM.tile([NDL, S], F32, tag="sexp")
            nc.scalar.activation(out=sexp, in_=srow, func=AF.Exp, bias=sneg)
            sem_t = SM.tile([NDL, S], F32, tag="sem")
            ssum = SM.tile([NDL, 1], F32, tag="ssum")
            nc.vector.tensor_tensor(out=sem_t, in0=sexp, in1=smask_sb, op=ALU.mult)
            nc.vector.tensor_reduce(out=ssum, in_=sem_t, axis=AX.X, op=ALU.add)
            sinv = SM.tile([NDL, 1], F32, tag="sinv")
            nc.vector.reciprocal(out=sinv, in_=ssum)
            sattn = SGL.tile([NDL, S], F32)
            nc.vector.tensor_scalar_mul(out=sattn, in0=sem_t, scalar1=sinv)
            nc.gpsimd.dma_start(out=out_sattn[:, :], in_=sattn)
            scols = []
            for h in range(2):
                pT4 = PP.tile([SH, NDL], F32, tag=f"pT4{h}")
                nc.tensor.transpose(
                    out=pT4[:, :], in_=sattn[:, h * SH:(h + 1) * SH],
                    identity=ident[:NDL, :NDL])
                scol_h = SGL.tile([SH, NDL], F32, tag=f"scol{h}")
                nc.vector.tensor_copy(out=scol_h, in_=pT4[:, :])
                scols.append(scol_h)

            PP_cm.__exit__(None, None, None)
            PP2_cm = tc.tile_pool(name="pp2", bufs=3, space="PSUM")
            PP2 = PP2_cm.__enter__()
            PCT_cm = tc.tile_pool(name="pct", bufs=2, space="PSUM")
            PCT = PCT_cm.__enter__()

            # ---------- word-level, per half-doc ----------
            for d in range(NDL):
                pct = PCT.tile([1, N], F32, tag="pct")
                eots = []
                for h in range(2):
                    w0 = d * WDOC + h * HALF_W
                    eot = EOP.tile([128, 10, N], BF16, tag="eot")
                    nc.gpsimd.dma_start(
                        out=eot,
                        in_=eo[w0:w0 + HALF_W, :].rearrange("(j p) n -> p j n", p=128),
                    )
                    eots.append(eot)
                for h in range(2):
                    w0 = d * WDOC + h * HALF_W
                    eft = EFP.tile([128, NCH, HALF_W], BF16, tag="eft")
                    nc.sync.dma_start(
                        out=eft,
                        in_=ef_T[:, w0:w0 + HALF_W].rearrange("(c p) w -> p c w", p=128),
                    )
                    wth = THP.tile([128, NCH, HALF_W], BF16, tag="wth")
                    for ci in range(NCH):
                        nc.scalar.activation(
                            out=wth[:, ci, :],
                            in_=eft[:, ci, :],
                            func=AF.Tanh,
                            bias=decb[:, ci, d:d + 1],
                        )
                    srow_h = SRP.tile([1, HALF_W], F32, tag="srow_h")
                    for g in range(NGRP):
                        psc = PP2.tile([1, GRP], F32, tag="psc")
                        for ci in range(NCH):
                            nc.tensor.matmul(
                                out=psc[:, :],
                                lhsT=v_bf[:, ci:ci + 1],
                                rhs=wth[:, ci, g * GRP:(g + 1) * GRP],
                                start=(ci == 0),
                                stop=(ci == NCH - 1),
                            )
                        nc.vector.tensor_copy(
                            out=srow_h[0:1, g * GRP:(g + 1) * GRP], in_=psc[:, :])
                    r = d * 2 + h
                    nc.gpsimd.dma_start(out=scr_w[r:r + 1, :], in_=srow_h)
                    scores_h = SM.tile([SH, TK], F32, tag="scores_h")
                    nc.gpsimd.dma_start(
                        out=scores_h[:, :],
                        in_=scr_w[r:r + 1, :].rearrange("o (s t) -> (o s) t", t=TK),
                    )

                    # per-half-doc softmax + combine with sentence attention
                    wm = M_img[:, r * TK:(r + 1) * TK]
                    neg = SM.tile([SH, 1], F32, tag="neg")
                    nc.vector.tensor_reduce(
                        out=neg, in_=scores_h, axis=AX.X, op=ALU.max, negate=True)
                    ex = SM.tile([SH, TK], F32, tag="ex")
                    nc.scalar.activation(out=ex, in_=scores_h, func=AF.Exp, bias=neg)
                    em = SM.tile([SH, TK], F32, tag="em")
                    wsum = SM.tile([SH, 1], F32, tag="wsum")
                    nc.vector.tensor_tensor(out=em, in0=ex, in1=wm, op=ALU.mult)
                    nc.vector.tensor_reduce(out=wsum, in_=em, axis=AX.X, op=ALU.add)
                    winv = SM.tile([SH, 1], F32, tag="winv")
                    nc.vector.reciprocal(out=winv, in_=wsum)
                    msc = SM.tile([SH, 1], F32, tag="msc")
                    nc.vector.tensor_mul(out=msc, in0=scols[h][:, d:d + 1], in1=winv)
                    ma = SM.tile([SH, TK], F32, tag="ma")
                    nc.vector.tensor_scalar_mul(out=ma, in0=em, scalar1=msc)

                    # per-sentence attention weights out; the host performs the
                    # ragged placement during unsharding (overlapping-run
                    # scatter is not expressible as a race-free device DMA)
                    r0 = d * S + h * SH
                    nc.gpsimd.dma_start(out=out_ma[r0:r0 + SH, :], in_=ma)

                    # weights for c_t: [128 words, 10 tiles] column layout
                    pT = PP2.tile([TK, SH], F32, tag="pT")
                    nc.tensor.transpose(
                        out=pT[:, :], in_=ma[:, :], identity=ident[:SH, :SH])
                    wcol = DOCP.tile([128, 10], BF16, tag="wcol")
                    nc.vector.tensor_copy(out=wcol[0:64, :], in_=pT[:, 0:SH:2])
                    nc.vector.tensor_copy(out=wcol[64:128, :], in_=pT[:, 1:SH:2])

                    # c_t accumulation: sum_w ma[w] * eo[w, :]
                    for j in range(10):
                        nc.tensor.matmul(
                            out=pct[:, :],
                            lhsT=wcol[:, j:j + 1],
                            rhs=eots[h][:, j, :],
                            start=(h == 0 and j == 0),
                            stop=(h == 1 and j == 9),
                        )
                ctsb = SM.tile([1, N], F32, tag="ctsb")
                nc.vector.tensor_copy(out=ctsb, in_=pct[:, :])
                nc.gpsimd.dma_start(out=out_ct[d:d + 1, :], in_=ctsb)

            PCT_cm.__exit__(None, None, None)
            PP2_cm.__exit__(None, None, None)

    _split_sync_waits(nc, limit=1)
    return nc


_NC = None


def _get_nc():
    global _NC
    if _NC is None:
        _NC = build_nc()
    return _NC


def _shard_inputs(s_t_hat, encoder_outputs, encoder_feature, seq_lens2, sent_s_t_hat,
                  sent_enc_outputs, sent_enc_feature, sent_enc_padding_mask, sent_lens,
                  max_doc_len, coverage, Wd, bd, v_w, Wsd, bsd, sv_w):
    f32 = np.float32
    B_SENT = encoder_outputs.shape[0]
    sent_lens = np.asarray(sent_lens, dtype=np.int64)
    seq_lens2 = np.asarray(seq_lens2, dtype=np.int64)
    sent_start = np.cumsum(sent_lens) - sent_lens            # [B_DOC]

    eo_full = np.ascontiguousarray(
        np.asarray(encoder_outputs, dtype=f32).reshape(B_SENT, TK, N))
    ef_full = np.ascontiguousarray(
        np.asarray(encoder_feature, dtype=f32).reshape(B_SENT, TK, N))
    sef_full = np.asarray(sent_enc_feature, dtype=f32).reshape(B_DOC, S, N)
    smask_full = np.asarray(sent_enc_padding_mask, dtype=f32)
    s_full = np.asarray(s_t_hat, dtype=f32)
    ss_full = np.asarray(sent_s_t_hat, dtype=f32)

    wd_img = np.asarray(Wd, dtype=f32).reshape(NCH, 128, N).transpose(1, 0, 2).reshape(128, NCH * N)
    wsd_img = np.asarray(Wsd, dtype=f32).reshape(NCH, 128, N).transpose(1, 0, 2).reshape(128, NCH * N)
    bd_img = np.asarray(bd, dtype=f32).reshape(NCH, 128).T
    bsd_img = np.asarray(bsd, dtype=f32).reshape(NCH, 128).T
    v_img = np.asarray(v_w, dtype=f32).reshape(NCH, 128).T
    sv_img = np.asarray(sv_w, dtype=f32).reshape(NCH, 128).T

    in_maps = []
    for c in range(NCORES):
        docs = list(range(c * NDL, (c + 1) * NDL))
        gi = np.zeros((NDL, S), dtype=np.int64)
        pad = np.zeros((NDL, S), dtype=bool)
        for dl, d in enumerate(docs):
            L = int(sent_lens[d])
            gi[dl, :L] = sent_start[d] + np.arange(L)
            pad[dl, L:] = True
        gflat = gi.reshape(-1)

        eo_sh = eo_full[gflat].reshape(NWL, N).astype(ml_dtypes.bfloat16)
        ef_sh = ef_full[gflat].reshape(NWL, N)
        ef_T = np.ascontiguousarray(ef_sh.T).astype(ml_dtypes.bfloat16)

        p1 = np.zeros((128, P1COLS), dtype=f32)
        p1[:, _C_WD:_C_ST] = wd_img
        sT = s_full[docs].T                                   # [N, NDL]
        p1[:, _C_ST:_C_BD] = sT.reshape(NCH, 128, NDL).transpose(1, 0, 2).reshape(128, NCH * NDL)
        p1[:, _C_BD:_C_V] = bd_img
        p1[:, _C_V:P1COLS] = v_img

        p2 = np.zeros((128, P2COLS), dtype=f32)
        p2[:, _C_WSD:_C_SST] = wsd_img
        ssT = ss_full[docs].T
        p2[:, _C_SST:_C_BSD] = ssT.reshape(NCH, 128, NDL).transpose(1, 0, 2).reshape(128, NCH * NDL)
        p2[:, _C_BSD:_C_SV] = bsd_img
        p2[:, _C_SV:_C_SEF] = sv_img
        sef_T = sef_full[docs].reshape(NSL, N).T              # [N, NSL]
        p2[:, _C_SEF:_C_SMASK] = sef_T.reshape(NCH, 128, NSL).transpose(1, 0, 2).reshape(128, NCH * NSL)
        p2[:4, _C_SMASK:_C_IDENT] = smask_full[docs].astype(f32)
        p2[:, _C_IDENT:P2COLS] = np.eye(128, dtype=f32)

        wlen = seq_lens2[gflat].reshape(NDL, S)
        wlen = np.where(pad, TK, wlen)
        wmask = (np.arange(TK)[None, None, :] < wlen[:, :, None]).astype(f32)
        wmask_img = np.ascontiguousarray(
            wmask.reshape(NDL, 2, SH, TK).transpose(2, 0, 1, 3).reshape(SH, NDL * 2 * TK))

        in_maps.append({
            "p1": p1,
            "p2": p2,
            "ef_T": ef_T,
            "eo": np.ascontiguousarray(eo_sh),
            "wmask": wmask_img,
        })
    return in_maps


def kernel(trace=False, **inputs):
    assert inputs["encoder_outputs"].shape == (B_DOC * S, TK, N)
    assert int(inputs["max_doc_len"]) == WDOC
    nc = _get_nc()
    in_maps = _shard_inputs(**inputs)
    res = run_bass_kernel_spmd(nc, in_maps, core_ids=list(range(NCORES)), trace=trace)
    c_t = np.concatenate([res.results[c]["out_ct"] for c in range(NCORES)], axis=0)
    sattn = np.concatenate([res.results[c]["out_sattn"] for c in range(NCORES)], axis=0)
    ma_all = np.concatenate(
        [res.results[c]["out_ma"].reshape(NDL, S, TK) for c in range(NCORES)], axis=0)

    # unshard: place each sentence's attention run at its ragged offset
    sent_lens = np.asarray(inputs["sent_lens"], dtype=np.int64)
    seq_lens2 = np.asarray(inputs["seq_lens2"], dtype=np.int64)
    B_SENT = seq_lens2.shape[0]
    sent_start = np.cumsum(sent_lens) - sent_lens
    word_cum = np.cumsum(seq_lens2) - seq_lens2
    doc_ids = np.repeat(np.arange(B_DOC), sent_lens)[:B_SENT]
    sent_local = np.arange(B_SENT) - sent_start[doc_ids]
    doc_word_off = word_cum - word_cum[sent_start[doc_ids]]
    pos = doc_word_off[:, None] + np.arange(TK)[None, :]
    valid = np.arange(TK)[None, :] < seq_lens2[:, None]
    flat = np.where(valid, doc_ids[:, None] * WDOC + pos, B_DOC * WDOC)
    vals = ma_all[doc_ids, sent_local]                     # [B_SENT, TK]
    attn = np.zeros(B_DOC * WDOC + 1, dtype=np.float32)
    attn[flat.reshape(-1)] = vals.reshape(-1)
    attn = attn[:B_DOC * WDOC].reshape(B_DOC, WDOC)

    coverage = np.asarray(inputs["coverage"], dtype=np.float32)
    out = (c_t, attn, coverage, sattn)
    if trace:
        return out, res
    return out


# revision 34
# speedup vs baseline: 1.1853x; 1.1853x over previous
"""Trainium2 Bass kernel for hierarchical (word + sentence) additive attention
with ragged per-document flattening.

Data-parallel over the 32-document batch: 4 docs per NeuronCore x 8 cores.
All parameters are replicated (packed into one image -> one DMA). Inputs are
sharded on the host; encoder_feature is pre-transposed to [N, words] bf16 and
encoder_outputs cast to bf16. Each core computes its 4 docs' outputs
(c_t rows, per-sentence attention rows, sentence-attention rows); the host
concatenates and performs the ragged placement of attention values during
unsharding.
"""

import numpy as np
import ml_dtypes

import concourse.bass as bass
from concourse import mybir
from concourse.tile import TileContext
from concourse.bass_utils import run_bass_kernel_spmd

F32 = mybir.dt.float32
BF16 = mybir.dt.bfloat16
FP8 = mybir.dt.float8e4
AF = mybir.ActivationFunctionType
ALU = mybir.AluOpType
AX = mybir.AxisListType

NCORES = 8
N = 512          # feature dim (2*hidden)
TK = 64          # max words per sentence
S = 40           # max sentences per doc
B_DOC = 32
NDL = B_DOC // NCORES      # docs per core = 4
NSL = NDL * S              # sentence slots per core = 160
WDOC = S * TK              # words per doc = 2560
NWL = NDL * WDOC           # words per core = 10240
NCH = N // 128             # 4 chunks of the feature dim
HALF_W = WDOC // 2         # 1280
SH = S // 2                # sentences per half doc = 20
GRP = 320                  # score-matmul word group (free dim)
NGRP = HALF_W // GRP       # 4 groups per half doc

# packed parameter images (f32), laid out per partition
# P1: word-level params (on the startup critical path)
_C_WD = 0
_C_ST = _C_WD + NCH * N           # 2048
_C_BD = _C_ST + NCH * NDL         # 2064
_C_V = _C_BD + NCH                # 2068
P1COLS = _C_V + NCH               # 2072
# P2: sentence-level params + identity
_C_WSD = 0
_C_SST = _C_WSD + NCH * N         # 2048
_C_BSD = _C_SST + NCH * NDL      # 2064
_C_SV = _C_BSD + NCH              # 2068
_C_SEF = _C_SV + NCH              # 2072
_C_SMASK = _C_SEF + NCH * NSL     # 2712
_C_IDENT = _C_SMASK + S           # 2752
P2COLS = _C_IDENT + 128           # 2880


def _patch_tile_drain():
    """This container's walrus build rejects >1 sync-wait on the Tile tail
    Drain (no-ctrl-struct instruction). Move the waits onto single-wait
    carrier NOPs on the sync engine."""
    import concourse.tile as tile_mod
    if getattr(tile_mod.TileContext, "_drain_patched", False):
        return
    from concourse.vector_clock import ScopedClock

    def _drain_and_barrier(self, tick_clock, wait_clock):
        nop = self.nc.sync.nop()
        wait_clock.add_sem_waits(nop.ins, ScopedClock({None: tick_clock.global_clock}))
        si = nop.ins.sync_info
        if si and si.on_wait and len(si.on_wait) > 1:
            extra = list(si.on_wait[1:])
            si.on_wait = si.on_wait[:1]
            for w in extra:
                n2 = self.nc.sync.nop()
                n2.ins.sync_info = mybir.SyncInfo(on_wait=[w], on_update=[])
        self.nc.sync.drain()
        self.nc.all_engine_barrier()
        popped = self.nc._tile_sem_poison_stack.pop()
        assert popped is self._sem_poison
        self.nc.clear_and_free_semaphores(list(self.sems.allocated().values()))
        self.nc.all_engine_barrier()

    tile_mod.TileContext._drain_and_barrier = _drain_and_barrier
    tile_mod.TileContext._drain_patched = True


def _split_sync_waits(nc, limit=1):
    """This container's walrus rejects instructions carrying more than `limit`
    sync waits. Hoist extra waits onto same-engine event-semaphore waits
    inserted just before the instruction (engine queues execute in order)."""
    for fn in nc.m.functions:
        for bb in fn.blocks:
            insts = bb.instructions
            i = 0
            while i < len(insts):
                inst = insts[i]
                si = inst.sync_info
                if si is not None and si.on_wait and len(si.on_wait) > limit:
                    waits = list(si.on_wait)
                    extra, keep = waits[:-limit], waits[-limit:]
                    for w in extra:
                        nop = mybir.InstEventSemaphore(
                            name=f"I-sw{nc.next_id()}",
                            engine=inst.engine,
                            sync_info=mybir.SyncInfo(on_wait=[w], on_update=[]),
                        )
                        insts.insert(i, nop)
                        i += 1
                    si.on_wait = keep
                i += 1


def build_nc():
    _patch_tile_drain()
    nc = bass.Bass(trn_type="TRN2")

    p1_d = nc.dram_tensor("p1", [128, P1COLS], F32, kind="ExternalInput")
    p2_d = nc.dram_tensor("p2", [128, P2COLS], F32, kind="ExternalInput")
    ef_T = nc.dram_tensor("ef_T", [N, NWL], FP8, kind="ExternalInput")
    eo = nc.dram_tensor("eo", [NWL, N], BF16, kind="ExternalInput")
    wmask_d = nc.dram_tensor("wmask", [SH, NDL * 2 * TK], F32, kind="ExternalInput")

    scr_s = nc.dram_tensor("scr_s", [1, NSL], F32, kind="Internal")
    scr_w = nc.dram_tensor("scr_w", [NDL * 2, HALF_W], F32, kind="Internal")

    out_ct = nc.dram_tensor("out_ct", [NDL, N], F32, kind="ExternalOutput")
    out_ma = nc.dram_tensor("out_ma", [NSL, TK], F32, kind="ExternalOutput")
    out_sattn = nc.dram_tensor("out_sattn", [NDL, S], F32, kind="ExternalOutput")

    with TileContext(nc) as tc:
        with (
            tc.tile_pool(name="singles", bufs=1) as SGL,
            tc.tile_pool(name="efp", bufs=4) as EFP,
            tc.tile_pool(name="thp", bufs=3) as THP,
            tc.tile_pool(name="eop", bufs=6) as EOP,
            tc.tile_pool(name="docp", bufs=3) as DOCP,
            tc.tile_pool(name="smalls", bufs=4) as SM,
            tc.tile_pool(name="srp", bufs=2) as SRP,
        ):
            # ---------- parameters: two packed images, two DMAs ----------
            P1 = SGL.tile([128, P1COLS], F32)
            nc.sync.dma_start(out=P1, in_=p1_d[:, :])
            P2 = SGL.tile([128, P2COLS], F32)
            nc.gpsimd.dma_start(out=P2, in_=p2_d[:, :])
            M_img = SGL.tile([SH, NDL * 2 * TK], F32)
            nc.sync.dma_start(out=M_img, in_=wmask_d[:, :])

            wd_sb = P1[:, _C_WD:_C_ST].rearrange("p (c n) -> p c n", n=N)
            sT_sb = P1[:, _C_ST:_C_BD].rearrange("p (c d) -> p c d", d=NDL)
            bd_sb = P1[:, _C_BD:_C_V]
            v_sb = P1[:, _C_V:P1COLS]
            wsd_sb = P2[:, _C_WSD:_C_SST].rearrange("p (c n) -> p c n", n=N)
            ssT_sb = P2[:, _C_SST:_C_BSD].rearrange("p (c d) -> p c d", d=NDL)
            bsd_sb = P2[:, _C_BSD:_C_SV]
            sv_sb = P2[:, _C_SV:_C_SEF]
            sefT_sb = P2[:, _C_SEF:_C_SMASK].rearrange("p (c s) -> p c s", s=NSL)
            smask_sb = P2[:4, _C_SMASK:_C_IDENT]
            ident = P2[:, _C_IDENT:P2COLS]

            v_bf = SGL.tile([128, NCH], BF16)
            nc.vector.tensor_copy(out=v_bf, in_=v_sb)

            # decoder features, transposed: decb[n_chunk][p, d] = (x @ W + b)[d, n]
            decb = SGL.tile([128, NCH, NDL], F32)
            sdecb = SGL.tile([128, NCH, NDL], F32)
            PP_cm = tc.tile_pool(name="pp1", bufs=1, space="PSUM")
            PP = PP_cm.__enter__()
            for w_sb, x_sb, b_sb, o_sb in (
                (wd_sb, sT_sb, bd_sb, decb),
                (wsd_sb, ssT_sb, bsd_sb, sdecb),
            ):
                for ci in range(NCH):
                    pdec = PP.tile([128, NDL], F32, tag="pdec")
                    for kc in range(NCH):
                        nc.tensor.matmul(
                            out=pdec[:, :],
                            lhsT=w_sb[:, kc, ci * 128:(ci + 1) * 128],
                            rhs=x_sb[:, kc, :],
                            start=(kc == 0),
                            stop=(kc == NCH - 1),
                        )
                    nc.vector.tensor_scalar_add(
                        out=o_sb[:, ci, :], in0=pdec[:, :], scalar1=b_sb[:, ci:ci + 1]
                    )

            # ---------- sentence-level attention ----------
            srow = SGL.tile([NDL, S], F32)
            stanh = SGL.tile([128, NCH, NSL], F32)
            for d in range(NDL):
                for ci in range(NCH):
                    nc.scalar.activation(
                        out=stanh[:, ci, d * S:(d + 1) * S],
                        in_=sefT_sb[:, ci, d * S:(d + 1) * S],
                        func=AF.Tanh,
                        bias=sdecb[:, ci, d:d + 1],
                    )
            pscs = PP.tile([1, NSL], F32, tag="pscs")
            for ci in range(NCH):
                nc.tensor.matmul(
                    out=pscs[:, :],
                    lhsT=sv_sb[:, ci:ci + 1],
                    rhs=stanh[:, ci, :],
                    start=(ci == 0),
                    stop=(ci == NCH - 1),
                )
            srow160 = SGL.tile([1, NSL], F32)
            nc.vector.tensor_copy(out=srow160[:, :], in_=pscs[0:1, :])
            nc.gpsimd.dma_start(out=scr_s[:, :], in_=srow160)
            nc.gpsimd.dma_start(
                out=srow[:, :], in_=scr_s.rearrange("o (d s) -> (o d) s", s=S))

            sneg = SM.tile([NDL, 1], F32, tag="sneg")
            nc.vector.tensor_reduce(out=sneg, in_=srow, axis=AX.X, op=ALU.max, negate=True)
            sexp = SM.tile([NDL, S], F32, tag="sexp")
            nc.scalar.activation(out=sexp, in_=srow, func=AF.Exp, bias=sneg)
            sem_t = SM.tile([NDL, S], F32, tag="sem")
            ssum = SM.tile([NDL, 1], F32, tag="ssum")
            nc.vector.tensor_tensor(out=sem_t, in0=sexp, in1=smask_sb, op=ALU.mult)
            nc.vector.tensor_reduce(out=ssum, in_=sem_t, axis=AX.X, op=ALU.add)
            sinv = SM.tile([NDL, 1], F32, tag="sinv")
            nc.vector.reciprocal(out=sinv, in_=ssum)
            sattn = SGL.tile([NDL, S], F32)
            nc.vector.tensor_scalar_mul(out=sattn, in0=sem_t, scalar1=sinv)
            nc.gpsimd.dma_start(out=out_sattn[:, :], in_=sattn)
            scols = []
            for h in range(2):
                pT4 = PP.tile([SH, NDL], F32, tag=f"pT4{h}")
                nc.tensor.transpose(
                    out=pT4[:, :], in_=sattn[:, h * SH:(h + 1) * SH],
                    identity=ident[:NDL, :NDL])
                scol_h = SGL.tile([SH, NDL], F32, tag=f"scol{h}")
                nc.vector.tensor_copy(out=scol_h, in_=pT4[:, :])
                scols.append(scol_h)

            PP_cm.__exit__(None, None, None)
            PP2_cm = tc.tile_pool(name="pp2", bufs=3, space="PSUM")
            PP2 = PP2_cm.__enter__()
            PCT_cm = tc.tile_pool(name="pct", bufs=2, space="PSUM")
            PCT = PCT_cm.__enter__()

            # ---------- word-level, per half-doc ----------
            for d in range(NDL):
                pct = PCT.tile([1, N], F32, tag="pct")
                eots = []
                for h in range(2):
                    w0 = d * WDOC + h * HALF_W
                    eot = EOP.tile([128, 10, N], BF16, tag="eot")
                    nc.gpsimd.dma_start(
                        out=eot,
                        in_=eo[w0:w0 + HALF_W, :].rearrange("(j p) n -> p j n", p=128),
                    )
                    eots.append(eot)
                for h in range(2):
                    w0 = d * WDOC + h * HALF_W
                    eft = EFP.tile([128, NCH, HALF_W], FP8, tag="eft")
                    nc.sync.dma_start(
                        out=eft,
                        in_=ef_T[:, w0:w0 + HALF_W].rearrange("(c p) w -> p c w", p=128),
                    )
                    wth = THP.tile([128, NCH, HALF_W], BF16, tag="wth")
                    for ci in range(NCH):
                        nc.scalar.activation(
                            out=wth[:, ci, :],
                            in_=eft[:, ci, :],
                            func=AF.Tanh,
                            bias=decb[:, ci, d:d + 1],
                        )
                    srow_h = SRP.tile([1, HALF_W], F32, tag="srow_h")
                    for g in range(NGRP):
                        psc = PP2.tile([1, GRP], F32, tag="psc")
                        for ci in range(NCH):
                            nc.tensor.matmul(
                                out=psc[:, :],
                                lhsT=v_bf[:, ci:ci + 1],
                                rhs=wth[:, ci, g * GRP:(g + 1) * GRP],
                                start=(ci == 0),
                                stop=(ci == NCH - 1),
                            )
                        nc.vector.tensor_copy(
                            out=srow_h[0:1, g * GRP:(g + 1) * GRP], in_=psc[:, :])
                    r = d * 2 + h
                    nc.gpsimd.dma_start(out=scr_w[r:r + 1, :], in_=srow_h)
                    scores_h = SM.tile([SH, TK], F32, tag="scores_h")
                    nc.gpsimd.dma_start(
                        out=scores_h[:, :],
                        in_=scr_w[r:r + 1, :].rearrange("o (s t) -> (o s) t", t=TK),
                    )

                    # per-half-doc softmax + combine with sentence attention
                    wm = M_img[:, r * TK:(r + 1) * TK]
                    neg = SM.tile([SH, 1], F32, tag="neg")
                    nc.vector.tensor_reduce(
                        out=neg, in_=scores_h, axis=AX.X, op=ALU.max, negate=True)
                    ex = SM.tile([SH, TK], F32, tag="ex")
                    nc.scalar.activation(out=ex, in_=scores_h, func=AF.Exp, bias=neg)
                    em = SM.tile([SH, TK], F32, tag="em")
                    wsum = SM.tile([SH, 1], F32, tag="wsum")
                    nc.vector.tensor_tensor(out=em, in0=ex, in1=wm, op=ALU.mult)
                    nc.vector.tensor_reduce(out=wsum, in_=em, axis=AX.X, op=ALU.add)
                    winv = SM.tile([SH, 1], F32, tag="winv")
                    nc.vector.reciprocal(out=winv, in_=wsum)
                    msc = SM.tile([SH, 1], F32, tag="msc")
                    nc.vector.tensor_mul(out=msc, in0=scols[h][:, d:d + 1], in1=winv)
                    ma = SM.tile([SH, TK], F32, tag="ma")
                    nc.vector.tensor_scalar_mul(out=ma, in0=em, scalar1=msc)

                    # per-sentence attention weights out; the host performs the
                    # ragged placement during unsharding (overlapping-run
                    # scatter is not expressible as a race-free device DMA)
                    r0 = d * S + h * SH
                    nc.gpsimd.dma_start(out=out_ma[r0:r0 + SH, :], in_=ma)

                    # weights for c_t: [128 words, 10 tiles] column layout
                    pT = PP2.tile([TK, SH], F32, tag="pT")
                    nc.tensor.transpose(
                        out=pT[:, :], in_=ma[:, :], identity=ident[:SH, :SH])
                    wcol = DOCP.tile([128, 10], BF16, tag="wcol")
                    nc.vector.tensor_copy(out=wcol[0:64, :], in_=pT[:, 0:SH:2])
                    nc.vector.tensor_copy(out=wcol[64:128, :], in_=pT[:, 1:SH:2])

                    # c_t accumulation: sum_w ma[w] * eo[w, :]
                    for j in range(10):
                        nc.tensor.matmul(
                            out=pct[:, :],
                            lhsT=wcol[:, j:j + 1],
                            rhs=eots[h][:, j, :],
                            start=(h == 0 and j == 0),
                            stop=(h == 1 and j == 9),
                        )
                ctsb = SM.tile([1, N], F32, tag="ctsb")
                nc.vector.tensor_copy(out=ctsb, in_=pct[:, :])
                nc.gpsimd.dma_start(out=out_ct[d:d + 1, :], in_=ctsb)

            PCT_cm.__exit__(None, None, None)
            PP2_cm.__exit__(None, None, None)

    _split_sync_waits(nc, limit=1)
    return nc


_NC = None


def _get_nc():
    global _NC
    if _NC is None:
        _NC = build_nc()
    return _NC


def _shard_inputs(s_t_hat, encoder_outputs, encoder_feature, seq_lens2, sent_s_t_hat,
                  sent_enc_outputs, sent_enc_feature, sent_enc_padding_mask, sent_lens,
                  max_doc_len, coverage, Wd, bd, v_w, Wsd, bsd, sv_w):
    f32 = np.float32
    B_SENT = encoder_outputs.shape[0]
    sent_lens = np.asarray(sent_lens, dtype=np.int64)
    seq_lens2 = np.asarray(seq_lens2, dtype=np.int64)
    sent_start = np.cumsum(sent_lens) - sent_lens            # [B_DOC]

    eo_full = np.ascontiguousarray(
        np.asarray(encoder_outputs, dtype=f32).reshape(B_SENT, TK, N))
    ef_full = np.ascontiguousarray(
        np.asarray(encoder_feature, dtype=f32).reshape(B_SENT, TK, N))
    sef_full = np.asarray(sent_enc_feature, dtype=f32).reshape(B_DOC, S, N)
    smask_full = np.asarray(sent_enc_padding_mask, dtype=f32)
    s_full = np.asarray(s_t_hat, dtype=f32)
    ss_full = np.asarray(sent_s_t_hat, dtype=f32)

    wd_img = np.asarray(Wd, dtype=f32).reshape(NCH, 128, N).transpose(1, 0, 2).reshape(128, NCH * N)
    wsd_img = np.asarray(Wsd, dtype=f32).reshape(NCH, 128, N).transpose(1, 0, 2).reshape(128, NCH * N)
    bd_img = np.asarray(bd, dtype=f32).reshape(NCH, 128).T
    bsd_img = np.asarray(bsd, dtype=f32).reshape(NCH, 128).T
    v_img = np.asarray(v_w, dtype=f32).reshape(NCH, 128).T
    sv_img = np.asarray(sv_w, dtype=f32).reshape(NCH, 128).T

    in_maps = []
    for c in range(NCORES):
        docs = list(range(c * NDL, (c + 1) * NDL))
        gi = np.zeros((NDL, S), dtype=np.int64)
        pad = np.zeros((NDL, S), dtype=bool)
        for dl, d in enumerate(docs):
            L = int(sent_lens[d])
            gi[dl, :L] = sent_start[d] + np.arange(L)
            pad[dl, L:] = True
        gflat = gi.reshape(-1)

        eo_sh = eo_full[gflat].reshape(NWL, N).astype(ml_dtypes.bfloat16)
        ef_sh = ef_full[gflat].reshape(NWL, N)
        ef_T = np.ascontiguousarray(ef_sh.T).astype(ml_dtypes.float8_e4m3)

        p1 = np.zeros((128, P1COLS), dtype=f32)
        p1[:, _C_WD:_C_ST] = wd_img
        sT = s_full[docs].T                                   # [N, NDL]
        p1[:, _C_ST:_C_BD] = sT.reshape(NCH, 128, NDL).transpose(1, 0, 2).reshape(128, NCH * NDL)
        p1[:, _C_BD:_C_V] = bd_img
        p1[:, _C_V:P1COLS] = v_img

        p2 = np.zeros((128, P2COLS), dtype=f32)
        p2[:, _C_WSD:_C_SST] = wsd_img
        ssT = ss_full[docs].T
        p2[:, _C_SST:_C_BSD] = ssT.reshape(NCH, 128, NDL).transpose(1, 0, 2).reshape(128, NCH * NDL)
        p2[:, _C_BSD:_C_SV] = bsd_img
        p2[:, _C_SV:_C_SEF] = sv_img
        sef_T = sef_full[docs].reshape(NSL, N).T              # [N, NSL]
        p2[:, _C_SEF:_C_SMASK] = sef_T.reshape(NCH, 128, NSL).transpose(1, 0, 2).reshape(128, NCH * NSL)
        p2[:4, _C_SMASK:_C_IDENT] = smask_full[docs].astype(f32)
        p2[:, _C_IDENT:P2COLS] = np.eye(128, dtype=f32)

        wlen = seq_lens2[gflat].reshape(NDL, S)
        wlen = np.where(pad, TK, wlen)
        wmask = (np.arange(TK)[None, None, :] < wlen[:, :, None]).astype(f32)
        wmask_img = np.ascontiguousarray(
            wmask.reshape(NDL, 2, SH, TK).transpose(2, 0, 1, 3).reshape(SH, NDL * 2 * TK))

        in_maps.append({
            "p1": p1,
            "p2": p2,
            "ef_T": ef_T,
            "eo": np.ascontiguousarray(eo_sh),
            "wmask": wmask_img,
        })
    return in_maps


def kernel(trace=False, **inputs):
    assert inputs["encoder_outputs"].shape == (B_DOC * S, TK, N)
    assert int(inputs["max_doc_len"]) == WDOC
    nc = _get_nc()
    in_maps = _shard_inputs(**inputs)
    res = run_bass_kernel_spmd(nc, in_maps, core_ids=list(range(NCORES)), trace=trace)
    c_t = np.concatenate([res.results[c]["out_ct"] for c in range(NCORES)], axis=0)
    sattn = np.concatenate([res.results[c]["out_sattn"] for c in range(NCORES)], axis=0)
    ma_all = np.concatenate(
        [res.results[c]["out_ma"].reshape(NDL, S, TK) for c in range(NCORES)], axis=0)

    # unshard: place each sentence's attention run at its ragged offset
    sent_lens = np.asarray(inputs["sent_lens"], dtype=np.int64)
    seq_lens2 = np.asarray(inputs["seq_lens2"], dtype=np.int64)
    B_SENT = seq_lens2.shape[0]
    sent_start = np.cumsum(sent_lens) - sent_lens
    word_cum = np.cumsum(seq_lens2) - seq_lens2
    doc_ids = np.repeat(np.arange(B_DOC), sent_lens)[:B_SENT]
    sent_local = np.arange(B_SENT) - sent_start[doc_ids]
    doc_word_off = word_cum - word_cum[sent_start[doc_ids]]
    pos = doc_word_off[:, None] + np.arange(TK)[None, :]
    valid = np.arange(TK)[None, :] < seq_lens2[:, None]
    flat = np.where(valid, doc_ids[:, None] * WDOC + pos, B_DOC * WDOC)
    vals = ma_all[doc_ids, sent_local]                     # [B_SENT, TK]
    attn = np.zeros(B_DOC * WDOC + 1, dtype=np.float32)
    attn[flat.reshape(-1)] = vals.reshape(-1)
    attn = attn[:B_DOC * WDOC].reshape(B_DOC, WDOC)

    coverage = np.asarray(inputs["coverage"], dtype=np.float32)
    out = (c_t, attn, coverage, sattn)
    if trace:
        return out, res
    return out
